# revision 3
# baseline (speedup 1.0000x reference)
"""Trainium2 Bass kernel for classical causal MHA (B=2, T=2048, D=1024, H=16).

Sharding: 8 cores = 2 batches x 4 head-groups (4 heads / 256 dims each).
Each core computes QKV projections for its head-group, causal attention,
and a partial output projection; the host sums the 4 partials per batch
and adds the (bv @ Wo + bo) correction (the v-bias commutes through
softmax-weighted averaging, so it is applied after the kernel).

All matmuls run as float32r (TF32-like fast path, 1 cycle/row at N>=256).
Attention S^T matmuls are K=64 row-packed pairs (two heads concurrently
on disjoint PE row groups). The softmax denominator comes from a ones
column appended to V in the PV matmul; no max-subtraction is needed
because scores are O(1) for this input distribution.
"""

import sys

sys.path.insert(0, "/opt/trn_rl_repo")

import numpy as np

D = 1024
NH = 16
HD = 64
NCORES = 8
GROUPS = 4          # head-groups per batch
HLOC = NH // GROUPS  # heads per core
CW = HLOC * HD       # per-core projection width (256)
SCALE = 1.0 / float(np.sqrt(HD))

_CACHE = {}


def build_nc(T):
    import concourse.tile as tile
    from concourse import bacc, mybir

    f32 = mybir.dt.float32
    f32r = mybir.dt.float32r
    AF = mybir.ActivationFunctionType

    QCH = min(512, T)     # q-chunk width for attention
    NQ = T // QCH
    RB = QCH // 128       # kblocks straddling one q-chunk
    KB = T // 128         # k/t blocks
    TB = T // 128
    NK = D // 128         # contraction chunks for projections
    PCH = min(512, T)     # q/k projection column chunk
    NP = T // PCH

    nc = bacc.Bacc(None, target_bir_lowering=False, debug=False)
    xT_d = nc.dram_tensor("xT", [D, T], f32, kind="ExternalInput")
    wq_d = nc.dram_tensor("wq", [D, CW], f32, kind="ExternalInput")
    wk_d = nc.dram_tensor("wk", [D, CW], f32, kind="ExternalInput")
    wv_d = nc.dram_tensor("wv", [D, CW], f32, kind="ExternalInput")
    wo_d = nc.dram_tensor("wo", [CW, D], f32, kind="ExternalInput")
    bq_d = nc.dram_tensor("bq2", [128, CW // 128], f32, kind="ExternalInput")
    bk_d = nc.dram_tensor("bk2", [128, CW // 128], f32, kind="ExternalInput")
    mask_d = nc.dram_tensor("maskT", [128, RB, QCH], f32, kind="ExternalInput")
    one_d = nc.dram_tensor("one1", [128, TB * HLOC], f32, kind="ExternalInput")
    out_d = nc.dram_tensor("out", [T, D], f32, kind="ExternalOutput")

    with tile.TileContext(nc) as tc:
        from contextlib import ExitStack

        with ExitStack() as es:
            pers = es.enter_context(tc.tile_pool(name="pers", bufs=1))
            psA = es.enter_context(tc.tile_pool(name="psA", bufs=4, space="PSUM"))
            psS = es.enter_context(tc.tile_pool(name="psS", bufs=2, space="PSUM"))
            psPV = es.enter_context(tc.tile_pool(name="psPV", bufs=1, space="PSUM"))

            qT_sb = pers.tile([128, 2, T], f32r, tag="qT")
            kT_sb = pers.tile([128, 2, T], f32r, tag="kT")
            attn_sb = pers.tile([128, 2, T], f32r, tag="attn")
            v1_sb = pers.tile([128, TB, HLOC, HD + 1], f32r, tag="v1")
            mask_sb = pers.tile([128, RB, QCH], f32r, tag="mask")
            bq_sb = pers.tile([128, CW // 128], f32, tag="bq")
            bk_sb = pers.tile([128, CW // 128], f32, tag="bk")
            wo_sb = pers.tile([128, 2, D], f32r, tag="wo")

            nc.sync.dma_start(mask_sb[:], mask_d[:].bitcast(f32r))
            nc.sync.dma_start(bq_sb[:], bq_d[:])
            nc.sync.dma_start(bk_sb[:], bk_d[:])
            nc.sync.dma_start(
                v1_sb[:, :, :, HD : HD + 1],
                one_d[:].rearrange("p (t h) -> p t h", h=HLOC).unsqueeze(-1).bitcast(f32r),
            )
            for m in range(2):
                nc.sync.dma_start(
                    wo_sb[:, m, :], wo_d[128 * m : 128 * (m + 1), :].bitcast(f32r)
                )

            with tc.tile_pool(name="xw", bufs=1) as xw:
                xT_sb = xw.tile([128, NK, T], f32r, tag="xT")
                wq_sb = xw.tile([128, NK, CW], f32r, tag="wq")
                wk_sb = xw.tile([128, NK, CW], f32r, tag="wk")
                wv_sb = xw.tile([128, NK, CW], f32r, tag="wv")
                for kk in range(NK):
                    sl = slice(128 * kk, 128 * (kk + 1))
                    nc.sync.dma_start(xT_sb[:, kk, :], xT_d[sl, :].bitcast(f32r))
                    nc.sync.dma_start(wv_sb[:, kk, :], wv_d[sl, :].bitcast(f32r))
                    nc.sync.dma_start(wq_sb[:, kk, :], wq_d[sl, :].bitcast(f32r))
                    nc.sync.dma_start(wk_sb[:, kk, :], wk_d[sl, :].bitcast(f32r))

                # ---- V projection (row layout, per 128-row block) ----
                for tb in range(TB):
                    pv = psA.tile([128, CW], mybir.dt.float32, tag="pa", name=f"pv{tb}")
                    for kk in range(NK):
                        nc.tensor.matmul(
                            pv[:],
                            xT_sb[:, kk, 128 * tb : 128 * (tb + 1)],
                            wv_sb[:, kk, :],
                            start=(kk == 0),
                            stop=(kk == NK - 1),
                        )
                    nc.vector.tensor_copy(
                        v1_sb[:, tb, :, 0:HD],
                        pv[:].rearrange("p (h d) -> p h d", h=HLOC),
                    )

                # ---- Q/K projections (transposed layout, head pairs) ----
                for m in range(2):
                    for dst, w_sb, b_sb in (
                        (qT_sb, wq_sb, bq_sb),
                        (kT_sb, wk_sb, bk_sb),
                    ):
                        for jc in range(NP):
                            pp = psA.tile(
                                [128, PCH], mybir.dt.float32, tag="pa",
                                name=f"pp{m}{jc}",
                            )
                            for kk in range(NK):
                                nc.tensor.matmul(
                                    pp[:],
                                    w_sb[:, kk, 128 * m : 128 * (m + 1)],
                                    xT_sb[:, kk, PCH * jc : PCH * (jc + 1)],
                                    start=(kk == 0),
                                    stop=(kk == NK - 1),
                                )
                            nc.scalar.activation(
                                dst[:, m, PCH * jc : PCH * (jc + 1)],
                                pp[:],
                                AF.Identity,
                                bias=b_sb[:, m : m + 1],
                                scale=1.0,
                            )

            # ---- attention ----
            # opened after the xw pool closes so they reuse its SBUF space
            ring = es.enter_context(tc.tile_pool(name="ring", bufs=6))
            small = es.enter_context(tc.tile_pool(name="small", bufs=2))
            LOOKAHEAD = 4
            for m in range(2):
                for j in range(NQ):
                    kb = (j + 1) * RB
                    q_sl = slice(QCH * j, QCH * (j + 1))
                    pvp = [
                        psPV.tile(
                            [128, QCH], mybir.dt.float32, tag=f"pvac{p}",
                            name=f"pvac{m}{j}{p}",
                        )
                        for p in range(2)
                    ]
                    pts = {}

                    def emit_s(i, m=m, j=j, q_sl=q_sl, pts=pts):
                        for p in range(2):
                            hsl = slice(64 * p, 64 * (p + 1))
                            sp = psS.tile(
                                [128, QCH], mybir.dt.float32, tag="s",
                                name=f"s{m}{j}{i}{p}",
                            )
                            pt = ring.tile(
                                [128, QCH], f32r, tag=f"pt{p}", name=f"pt{m}{j}{i}{p}"
                            )
                            nc.tensor.matmul(
                                sp[:],
                                kT_sb[hsl, m, 128 * i : 128 * (i + 1)],
                                qT_sb[hsl, m, q_sl],
                                start=True,
                                stop=True,
                            )
                            nc.scalar.activation(pt[:], sp[:], AF.Exp, scale=SCALE)
                            if i >= RB * j:
                                nc.vector.tensor_mul(
                                    pt[:], pt[:], mask_sb[:, i - RB * j, :]
                                )
                            pts[(i, p)] = pt

                    def emit_pv(i, m=m, kb=kb, pvp=pvp, pts=pts):
                        for p in range(2):
                            nc.tensor.matmul(
                                pvp[p][0 : HD + 1, :],
                                v1_sb[:, i, 2 * m + p, :],
                                pts[(i, p)][:],
                                start=(i == 0),
                                stop=(i == kb - 1),
                            )

                    for i in range(kb):
                        emit_s(i)
                        if i >= LOOKAHEAD:
                            emit_pv(i - LOOKAHEAD)
                    for i in range(max(0, kb - LOOKAHEAD), kb):
                        emit_pv(i)

                    for p in range(2):
                        recip = small.tile([1, QCH], f32, tag="recip",
                                           name=f"rc{m}{j}{p}")
                        nc.vector.reciprocal(recip[:], pvp[p][HD : HD + 1, :])
                        bcast = small.tile([64, QCH], f32, tag="bcast", bufs=3,
                                           name=f"bc{m}{j}{p}")
                        nc.gpsimd.partition_broadcast(bcast[:], recip[:])
                        if p == 0:
                            nc.vector.tensor_mul(
                                attn_sb[0:64, m, q_sl], pvp[p][0:HD, :], bcast[:]
                            )
                        else:
                            tmp = small.tile([64, QCH], f32r, tag="tmp",
                                             name=f"tmp{m}{j}")
                            nc.vector.tensor_mul(tmp[:], pvp[p][0:HD, :], bcast[:])
                            nc.sync.dma_start(attn_sb[64:128, m, q_sl], tmp[:])

            # ---- output projection (partial, host sums across cores) ----
            for tb in range(TB):
                t_sl = slice(128 * tb, 128 * (tb + 1))
                o_sb = small.tile([128, D], f32, tag="osb", name=f"osb{tb}")
                for n in range(D // 512):
                    po = psA.tile([128, 512], mybir.dt.float32, tag="pa",
                                  name=f"po{tb}{n}")
                    for m in range(2):
                        nc.tensor.matmul(
                            po[:],
                            attn_sb[:, m, t_sl],
                            wo_sb[:, m, 512 * n : 512 * (n + 1)],
                            start=(m == 0),
                            stop=(m == 1),
                        )
                    nc.vector.tensor_copy(o_sb[:, 512 * n : 512 * (n + 1)], po[:])
                nc.sync.dma_start(out_d[t_sl, :], o_sb[:])

    nc.compile()
    return nc


def make_aux(T):
    QCH = min(512, T)
    RB = QCH // 128
    TB = T // 128
    kk = np.arange(128)[:, None, None]
    ri = np.arange(RB)[None, :, None]
    qq = np.arange(QCH)[None, None, :]
    mask = (qq >= kk + 128 * ri).astype(np.float32)
    ones = np.ones((128, TB * HLOC), np.float32)
    return mask, ones


def shard_inputs(x, Wq, bq, Wk, bk, Wv, Wo):
    T = x.shape[1]
    mask, ones = make_aux(T)
    in_maps = []
    for c in range(NCORES):
        b, g = divmod(c, GROUPS)
        cols = slice(g * CW, (g + 1) * CW)
        in_maps.append(
            {
                "xT": np.ascontiguousarray(x[b].T),
                "wq": np.ascontiguousarray(Wq[:, cols]),
                "wk": np.ascontiguousarray(Wk[:, cols]),
                "wv": np.ascontiguousarray(Wv[:, cols]),
                "wo": np.ascontiguousarray(Wo[cols, :]),
                "bq2": np.ascontiguousarray(bq[cols].reshape(CW // 128, 128).T),
                "bk2": np.ascontiguousarray(bk[cols].reshape(CW // 128, 128).T),
                "maskT": mask,
                "one1": ones,
            }
        )
    return in_maps


def gather_outputs(results, x, Wv_b, Wo, bo, bv):
    B, T, _ = x.shape
    y = np.empty((B, T, D), np.float32)
    corr = (bv @ Wo + bo).astype(np.float32)
    for b in range(B):
        acc = results[GROUPS * b]["out"].copy()
        for g in range(1, GROUPS):
            acc += results[GROUPS * b + g]["out"]
        y[b] = acc + corr
    return y


def kernel(x, Wq, bq, Wk, bk, Wv, bv, Wo, bo, _trace=False):
    from concourse import bass_utils

    x = np.asarray(x, np.float32)
    T = x.shape[1]
    if T not in _CACHE:
        _CACHE[T] = build_nc(T)
    nc = _CACHE[T]
    in_maps = shard_inputs(
        x,
        np.asarray(Wq, np.float32), np.asarray(bq, np.float32),
        np.asarray(Wk, np.float32), np.asarray(bk, np.float32),
        np.asarray(Wv, np.float32), np.asarray(Wo, np.float32),
    )
    res = bass_utils.run_bass_kernel_spmd(
        nc, in_maps, core_ids=list(range(NCORES)), trace=_trace
    )
    y = gather_outputs(res.results, x, None, np.asarray(Wo, np.float32),
                       np.asarray(bo, np.float32), np.asarray(bv, np.float32))
    if _trace:
        return y, res
    return y
